# revision 24
# baseline (speedup 1.0000x reference)
"""Behler-Parrinello NN potential kernel for 8x Trainium2 NeuronCores.

Strategy (moe_routing — route instead of dense-compute-both-experts):
  - Host: partition atoms by type, pad each per-core type group to a
    multiple of 128, shard across 8 cores (data-parallel over atoms,
    per-type MLP weights replicated). Each core gets a feature-major
    (transposed) [128, n_per_core] fp16 slice of Gs, so the contraction
    dim always sits on SBUF partitions and activations stay
    feature-major through all 3 layers — no on-device transposes.
  - Device (per core, Bass/Tile): per 1024-atom block run the 3-layer
    MLP of the block's expert. fp16 operands at full PE rate (1
    cycle/row) with fp32 PSUM accumulation; tanh+bias fused on the
    scalar engine reading PSUM; emitted as a 3-stage software pipeline
    (L1(b) | L2(b-1) | L3(b-2)) so the in-order PE stream never waits
    on the tanh chain. The M=1 output layer is packed 4-up into
    distinct 32-column PE groups (tile_position) to run concurrently.
  - Host: sum the 4 output-layer partial rows, add b3+offset, scatter
    energies back to original atom order, segment-mean per molecule
    with bincount (~0.0001% of the FLOPs).
"""

import sys
import time

sys.path.insert(0, "/opt/trn_rl_repo")

import numpy as np

import concourse.bacc as bacc
import concourse.mybir as mybir
from concourse import tile
from concourse.bass_utils import run_bass_kernel_spmd

N_CORES = 8
NUM_GS = 128
HIDDEN = 512
N_MOL = 1024
BLK = 1024          # atoms per block (PSUM tile = 2 banks)
MCH = HIDDEN // 128  # hidden chunks of 128 (partition tiles)

F32 = mybir.dt.float32
F16 = mybir.dt.float16
# fp16 operands: full PE rate (1 cycle/row), 1024-wide moving operand,
# fast weight load, half the DMA/SBUF of fp32. PSUM accumulates in fp32.
MM_DT = mybir.dt.float16
MCHUNK = 512        # moving-operand chunk (1 PSUM bank out)

_PROGRAM_CACHE: dict = {}


def _build_program(n_a: int, n_b: int):
    """Build the SPMD Bass program: n_a A-atoms + n_b B-atoms per core
    (both multiples of 128)."""
    key = (n_a, n_b, str(MM_DT))
    if key in _PROGRAM_CACHE:
        return _PROGRAM_CACHE[key]

    ntot = n_a + n_b
    nc = bacc.Bacc("TRN2", target_bir_lowering=False, debug=False,
                   num_devices=N_CORES)

    gst = nc.dram_tensor("gst", [NUM_GS, ntot], MM_DT, kind="ExternalInput")
    e_out = nc.dram_tensor("e_out", [MCH, ntot], F32, kind="ExternalOutput")
    dram = {}
    for t in ("a", "b"):
        dram[f"w1{t}"] = nc.dram_tensor(f"w1{t}", [NUM_GS, HIDDEN], MM_DT,
                                        kind="ExternalInput")
        dram[f"w2{t}"] = nc.dram_tensor(f"w2{t}", [HIDDEN, HIDDEN], MM_DT,
                                        kind="ExternalInput")
        dram[f"w3{t}"] = nc.dram_tensor(f"w3{t}", [128, 32 * MCH], F16,
                                        kind="ExternalInput")
        dram[f"b1{t}"] = nc.dram_tensor(f"b1{t}", [128, MCH], F32,
                                        kind="ExternalInput")
        dram[f"b2{t}"] = nc.dram_tensor(f"b2{t}", [128, MCH], F32,
                                        kind="ExternalInput")

    Tanh = mybir.ActivationFunctionType.Tanh

    with tile.TileContext(nc) as tc:
        with (
            tc.tile_pool(name="wpool", bufs=1) as wpool,
            tc.tile_pool(name="gpool", bufs=3) as gpool,
            tc.tile_pool(name="h1pool", bufs=10) as h1pool,
            tc.tile_pool(name="h2pool", bufs=10) as h2pool,
            tc.tile_pool(name="epool", bufs=3) as epool,
            tc.tile_pool(name="pspool", bufs=4, space="PSUM") as pspool,
        ):
            # Warm the PE (HAM clock gate) with matmuls on scratch SBUF
            # while the first DMAs are still in flight; result never read.
            scratch = wpool.tile([128, MCHUNK], MM_DT, tag="scratch")
            nc.gpsimd.memset(scratch[:, :], 0)
            wps = pspool.tile([128, MCHUNK], F32, tag="ps")
            for i in range(16):
                nc.tensor.matmul(wps[:, :], scratch[:, 0:128], scratch[:, :],
                                 start=(i == 0), stop=(i == 15))

            # Most weights go on the gpsimd DMA queue so the first gs
            # block (sync queue) isn't stuck behind weight traffic; w1 is
            # needed first, so it leads the sync queue.
            sb = {}
            for t in ("a", "b"):
                w1 = wpool.tile([128, HIDDEN], MM_DT, tag=f"w1{t}")
                eng = nc.sync if t == "a" else nc.gpsimd
                eng.dma_start(w1[:, :], dram[f"w1{t}"][:, :])
                b1 = wpool.tile([128, MCH], F32, tag=f"b1{t}")
                nc.gpsimd.dma_start(b1[:, :], dram[f"b1{t}"][:, :])
                w2 = []
                for k in range(MCH):
                    w2k = wpool.tile([128, HIDDEN], MM_DT, tag=f"w2{t}{k}")
                    nc.gpsimd.dma_start(
                        w2k[:, :], dram[f"w2{t}"][k * 128:(k + 1) * 128, :])
                    w2.append(w2k)
                w3 = wpool.tile([128, 32 * MCH], F16, tag=f"w3{t}")
                nc.gpsimd.dma_start(w3[:, :], dram[f"w3{t}"][:, :])
                b2 = wpool.tile([128, MCH], F32, tag=f"b2{t}")
                nc.gpsimd.dma_start(b2[:, :], dram[f"b2{t}"][:, :])
                sb[t] = (w1, w2, w3, b1, b2)

            # Block schedule: contiguous A atoms, then B atoms.
            blocks = []
            off = 0
            for t, n_at in (("a", n_a), ("b", n_b)):
                rem = n_at
                while rem:
                    w = min(BLK, rem)
                    blocks.append((t, off, w))
                    off += w
                    rem -= w

            h1_of, h2_of = {}, {}

            def chunks_of(w):
                out, c0 = [], 0
                while c0 < w:
                    cw = min(MCHUNK, w - c0)
                    out.append((c0, cw))
                    c0 += cw
                return out

            def emit_l1(bi):
                ex, off, w = blocks[bi]
                w1, _, _, b1, _ = sb[ex]
                gs = gpool.tile([128, w], MM_DT, tag="gs")
                nc.sync.dma_start(gs[:, :], gst[:, off:off + w])
                h1 = []
                for m in range(MCH):
                    ps = pspool.tile([128, w], F32, tag="ps")
                    lhs = w1[:, m * 128:(m + 1) * 128]
                    for c0, cw in chunks_of(w):
                        nc.tensor.matmul(
                            ps[:, c0:c0 + cw], lhs,
                            gs[:, c0:c0 + cw],
                            start=True, stop=True)
                    h1m = h1pool.tile([128, w], MM_DT, tag="h1")
                    nc.scalar.activation(h1m[:, :], ps[:, :], Tanh,
                                         bias=b1[:, m:m + 1], scale=1.0)
                    h1.append(h1m)
                h1_of[bi] = h1

            def emit_l2(bi):
                ex, off, w = blocks[bi]
                _, w2, _, _, b2 = sb[ex]
                h1 = h1_of.pop(bi)
                h2 = []
                for m in range(MCH):
                    ps = pspool.tile([128, w], F32, tag="ps")
                    for k in range(MCH):
                        lhs = w2[k][:, m * 128:(m + 1) * 128]
                        for c0, cw in chunks_of(w):
                            nc.tensor.matmul(
                                ps[:, c0:c0 + cw], lhs,
                                h1[k][:, c0:c0 + cw],
                                start=(k == 0), stop=(k == MCH - 1))
                    h2m = h2pool.tile([128, w], F16, tag="h2")
                    nc.scalar.activation(h2m[:, :], ps[:, :], Tanh,
                                         bias=b2[:, m:m + 1], scale=1.0)
                    h2.append(h2m)
                h2_of[bi] = h2

            def emit_l3(bi):
                # M=1 matmuls packed 4-up in distinct 32-column PE groups
                # (tile_position) so the 4 kin-chunks run concurrently.
                # The 4 partial rows land on psum partitions 0/32/64/96 and
                # are summed on the host during the unshard.
                ex, off, w = blocks[bi]
                _, _, w3, _, _ = sb[ex]
                h2 = h2_of.pop(bi)
                ps3 = pspool.tile([128, w], F32, tag="ps")
                for c0, cw in chunks_of(w):
                    for k in range(MCH):
                        nc.tensor.matmul(
                            ps3[32 * k:32 * (k + 1), c0:c0 + cw],
                            w3[:, 32 * k:32 * (k + 1)],
                            h2[k][:, c0:c0 + cw],
                            start=True, stop=True,
                            tile_position=(0, 32 * k))
                e_sb = epool.tile([97, w], F32, tag="e")
                nc.vector.tensor_copy(e_sb[:, :], ps3[0:97, :])
                nc.sync.dma_start(e_out[:, off:off + w], e_sb[0:97:32, :])

            # 3-stage software pipeline: L1(b) || L2(b-1) || L3(b-2) so
            # the in-order PE stream never waits on the tanh (ACT) chain.
            nblocks = len(blocks)
            for i in range(nblocks + 2):
                if i < nblocks:
                    emit_l1(i)
                if 0 <= i - 1 < nblocks:
                    emit_l2(i - 1)
                if 0 <= i - 2 < nblocks:
                    emit_l3(i - 2)

    nc.compile()
    _PROGRAM_CACHE[key] = nc
    return nc


def kernel(**inputs) -> np.ndarray:
    Gs = np.ascontiguousarray(np.asarray(inputs["Gs"], dtype=np.float32))
    types = np.asarray(inputs["types"])
    mol_id = np.asarray(inputs["mol_id"])
    n_atoms = Gs.shape[0]

    idx = [np.flatnonzero(types == 0), np.flatnonzero(types != 0)]
    # Per-core atom counts (equal across cores for SPMD; pad with zeros).
    GRAN = 128
    n_a, n_b = (int(-(-len(ix) // (N_CORES * GRAN))) * GRAN for ix in idx)
    npc = n_a + n_b

    GsT = Gs.astype(np.float16).T  # [128, N] fp16 view

    in_maps = []
    wk = {}
    for t, pre in (("a", "A"), ("b", "B")):
        wk[f"w1{t}"] = np.ascontiguousarray(
            np.asarray(inputs[f"W1_{pre}"], np.float32).astype(np.float16))
        wk[f"w2{t}"] = np.ascontiguousarray(
            np.asarray(inputs[f"W2_{pre}"], np.float32).astype(np.float16))
        w3chunks = np.asarray(
            inputs[f"W3_{pre}"], np.float32)[:, 0].reshape(MCH, 128).T
        w3p = np.zeros((128, 32 * MCH), np.float16)
        w3p[:, 0::32] = w3chunks.astype(np.float16)
        wk[f"w3{t}"] = w3p
        wk[f"b1{t}"] = np.ascontiguousarray(
            np.asarray(inputs[f"b1_{pre}"], np.float32).reshape(MCH, 128).T)
        wk[f"b2{t}"] = np.ascontiguousarray(
            np.asarray(inputs[f"b2_{pre}"], np.float32).reshape(MCH, 128).T)
        wk[f"b3{t}"] = np.float32(
            np.asarray(inputs[f"b3_{pre}"], np.float32).reshape(())
            + np.asarray(inputs[f"off_{pre}"], np.float32).reshape(()))

    chunks = []  # per core: (a_indices, b_indices)
    for i in range(N_CORES):
        ca = idx[0][i * n_a:(i + 1) * n_a]
        cb = idx[1][i * n_b:(i + 1) * n_b]
        chunks.append((ca, cb))
        buf = np.zeros((NUM_GS, npc), np.float16)
        buf[:, :len(ca)] = GsT[:, ca]
        buf[:, n_a:n_a + len(cb)] = GsT[:, cb]
        in_maps.append({"gst": buf,
                        **{k: v for k, v in wk.items()
                           if not k.startswith("b3")}})

    nc = _build_program(n_a, n_b)
    results = None
    for attempt in range(3):
        try:
            results = run_bass_kernel_spmd(
                nc, in_maps, list(range(N_CORES))).results
            break
        except Exception:
            # Transient NRT/device hiccups (e.g. NRT_EXEC_UNIT_UNRECOVERABLE)
            # usually clear on retry.
            if attempt == 2:
                raise
            time.sleep(2.0)

    e = np.empty(n_atoms, np.float32)
    for i in range(N_CORES):
        r = np.asarray(results[i]["e_out"]).sum(axis=0, dtype=np.float32)
        ca, cb = chunks[i]
        e[ca] = r[:len(ca)] + wk["b3a"]
        e[cb] = r[n_a:n_a + len(cb)] + wk["b3b"]

    sums = np.bincount(mol_id, weights=e.astype(np.float64),
                       minlength=N_MOL)[:N_MOL]
    counts = np.bincount(mol_id, minlength=N_MOL)[:N_MOL]
    out = sums / np.maximum(counts, 1)
    return out.astype(np.float32)[:, None]


# revision 25
# speedup vs baseline: 1.0019x; 1.0019x over previous
"""Behler-Parrinello NN potential kernel for 8x Trainium2 NeuronCores.

Strategy (moe_routing — route instead of dense-compute-both-experts):
  - Host: partition atoms by type, pad each per-core type group to a
    multiple of 128, shard across 8 cores (data-parallel over atoms,
    per-type MLP weights replicated). Each core gets a feature-major
    (transposed) [128, n_per_core] fp16 slice of Gs, so the contraction
    dim always sits on SBUF partitions and activations stay
    feature-major through all 3 layers — no on-device transposes.
  - Device (per core, Bass/Tile): per 1024-atom block run the 3-layer
    MLP of the block's expert. fp16 operands at full PE rate (1
    cycle/row) with fp32 PSUM accumulation; tanh+bias fused on the
    scalar engine reading PSUM; emitted as a 3-stage software pipeline
    (L1(b) | L2(b-1) | L3(b-2)) so the in-order PE stream never waits
    on the tanh chain. The M=1 output layer is packed 4-up into
    distinct 32-column PE groups (tile_position) to run concurrently.
  - Host: sum the 4 output-layer partial rows, add b3+offset, scatter
    energies back to original atom order, segment-mean per molecule
    with bincount (~0.0001% of the FLOPs).
"""

import sys
import time

sys.path.insert(0, "/opt/trn_rl_repo")

import numpy as np

import concourse.bacc as bacc
import concourse.mybir as mybir
from concourse import tile
from concourse.bass_utils import run_bass_kernel_spmd

N_CORES = 8
NUM_GS = 128
HIDDEN = 512
N_MOL = 1024
BLK = 1024          # atoms per block (PSUM tile = 2 banks)
MCH = HIDDEN // 128  # hidden chunks of 128 (partition tiles)

F32 = mybir.dt.float32
F16 = mybir.dt.float16
# fp16 operands: full PE rate (1 cycle/row), 1024-wide moving operand,
# fast weight load, half the DMA/SBUF of fp32. PSUM accumulates in fp32.
MM_DT = mybir.dt.float16
MCHUNK = 512        # moving-operand chunk (1 PSUM bank out)

_PROGRAM_CACHE: dict = {}


def _build_program(n_a: int, n_b: int):
    """Build the SPMD Bass program: n_a A-atoms + n_b B-atoms per core
    (both multiples of 128)."""
    key = (n_a, n_b, str(MM_DT))
    if key in _PROGRAM_CACHE:
        return _PROGRAM_CACHE[key]

    ntot = n_a + n_b
    nc = bacc.Bacc("TRN2", target_bir_lowering=False, debug=False,
                   num_devices=N_CORES)

    gst = nc.dram_tensor("gst", [NUM_GS, ntot], MM_DT, kind="ExternalInput")
    e_out = nc.dram_tensor("e_out", [MCH, ntot], F32, kind="ExternalOutput")
    dram = {}
    for t in ("a", "b"):
        dram[f"w1{t}"] = nc.dram_tensor(f"w1{t}", [NUM_GS, HIDDEN], MM_DT,
                                        kind="ExternalInput")
        dram[f"w2{t}"] = nc.dram_tensor(f"w2{t}", [HIDDEN, HIDDEN], MM_DT,
                                        kind="ExternalInput")
        dram[f"w3{t}"] = nc.dram_tensor(f"w3{t}", [128, 32 * MCH], F16,
                                        kind="ExternalInput")
        dram[f"b1{t}"] = nc.dram_tensor(f"b1{t}", [128, MCH], F32,
                                        kind="ExternalInput")
        dram[f"b2{t}"] = nc.dram_tensor(f"b2{t}", [128, MCH], F32,
                                        kind="ExternalInput")

    Tanh = mybir.ActivationFunctionType.Tanh

    with tile.TileContext(nc) as tc:
        with (
            tc.tile_pool(name="wpool", bufs=1) as wpool,
            tc.tile_pool(name="gpool", bufs=4) as gpool,
            tc.tile_pool(name="h1pool", bufs=12) as h1pool,
            tc.tile_pool(name="h2pool", bufs=12) as h2pool,
            tc.tile_pool(name="epool", bufs=3) as epool,
            tc.tile_pool(name="pspool", bufs=4, space="PSUM") as pspool,
        ):
            # Warm the PE (HAM clock gate) with matmuls on scratch SBUF
            # while the first DMAs are still in flight; result never read.
            scratch = wpool.tile([128, MCHUNK], MM_DT, tag="scratch")
            nc.gpsimd.memset(scratch[:, :], 0)
            wps = pspool.tile([128, MCHUNK], F32, tag="ps")
            for i in range(16):
                nc.tensor.matmul(wps[:, :], scratch[:, 0:128], scratch[:, :],
                                 start=(i == 0), stop=(i == 15))

            # Most weights go on the gpsimd DMA queue so the first gs
            # block (sync queue) isn't stuck behind weight traffic; w1 is
            # needed first, so it leads the sync queue.
            sb = {}
            for t in ("a", "b"):
                w1 = wpool.tile([128, HIDDEN], MM_DT, tag=f"w1{t}")
                eng = nc.sync if t == "a" else nc.gpsimd
                eng.dma_start(w1[:, :], dram[f"w1{t}"][:, :])
                b1 = wpool.tile([128, MCH], F32, tag=f"b1{t}")
                nc.gpsimd.dma_start(b1[:, :], dram[f"b1{t}"][:, :])
                w2 = []
                for k in range(MCH):
                    w2k = wpool.tile([128, HIDDEN], MM_DT, tag=f"w2{t}{k}")
                    nc.gpsimd.dma_start(
                        w2k[:, :], dram[f"w2{t}"][k * 128:(k + 1) * 128, :])
                    w2.append(w2k)
                w3 = wpool.tile([128, 32 * MCH], F16, tag=f"w3{t}")
                nc.gpsimd.dma_start(w3[:, :], dram[f"w3{t}"][:, :])
                b2 = wpool.tile([128, MCH], F32, tag=f"b2{t}")
                nc.gpsimd.dma_start(b2[:, :], dram[f"b2{t}"][:, :])
                sb[t] = (w1, w2, w3, b1, b2)

            # Block schedule: contiguous A atoms, then B atoms.
            blocks = []
            off = 0
            for t, n_at in (("a", n_a), ("b", n_b)):
                rem = n_at
                while rem:
                    w = min(BLK, rem)
                    blocks.append((t, off, w))
                    off += w
                    rem -= w

            h1_of, h2_of = {}, {}

            def chunks_of(w):
                out, c0 = [], 0
                while c0 < w:
                    cw = min(MCHUNK, w - c0)
                    out.append((c0, cw))
                    c0 += cw
                return out

            def emit_l1(bi):
                ex, off, w = blocks[bi]
                w1, _, _, b1, _ = sb[ex]
                gs = gpool.tile([128, w], MM_DT, tag="gs")
                for c0, cw in chunks_of(w):
                    nc.sync.dma_start(gs[:, c0:c0 + cw],
                                      gst[:, off + c0:off + c0 + cw])
                h1 = []
                for m in range(MCH):
                    ps = pspool.tile([128, w], F32, tag="ps")
                    lhs = w1[:, m * 128:(m + 1) * 128]
                    for c0, cw in chunks_of(w):
                        nc.tensor.matmul(
                            ps[:, c0:c0 + cw], lhs,
                            gs[:, c0:c0 + cw],
                            start=True, stop=True)
                    h1m = h1pool.tile([128, w], MM_DT, tag="h1")
                    nc.scalar.activation(h1m[:, :], ps[:, :], Tanh,
                                         bias=b1[:, m:m + 1], scale=1.0)
                    h1.append(h1m)
                h1_of[bi] = h1

            def emit_l2(bi):
                ex, off, w = blocks[bi]
                _, w2, _, _, b2 = sb[ex]
                h1 = h1_of.pop(bi)
                h2 = []
                for m in range(MCH):
                    ps = pspool.tile([128, w], F32, tag="ps")
                    for k in range(MCH):
                        lhs = w2[k][:, m * 128:(m + 1) * 128]
                        for c0, cw in chunks_of(w):
                            nc.tensor.matmul(
                                ps[:, c0:c0 + cw], lhs,
                                h1[k][:, c0:c0 + cw],
                                start=(k == 0), stop=(k == MCH - 1))
                    h2m = h2pool.tile([128, w], F16, tag="h2")
                    nc.scalar.activation(h2m[:, :], ps[:, :], Tanh,
                                         bias=b2[:, m:m + 1], scale=1.0)
                    h2.append(h2m)
                h2_of[bi] = h2

            def emit_l3(bi):
                # M=1 matmuls packed 4-up in distinct 32-column PE groups
                # (tile_position) so the 4 kin-chunks run concurrently.
                # The 4 partial rows land on psum partitions 0/32/64/96 and
                # are summed on the host during the unshard.
                ex, off, w = blocks[bi]
                _, _, w3, _, _ = sb[ex]
                h2 = h2_of.pop(bi)
                ps3 = pspool.tile([128, w], F32, tag="ps")
                for c0, cw in chunks_of(w):
                    for k in range(MCH):
                        nc.tensor.matmul(
                            ps3[32 * k:32 * (k + 1), c0:c0 + cw],
                            w3[:, 32 * k:32 * (k + 1)],
                            h2[k][:, c0:c0 + cw],
                            start=True, stop=True,
                            tile_position=(0, 32 * k))
                e_sb = epool.tile([97, w], F32, tag="e")
                nc.vector.tensor_copy(e_sb[:, :], ps3[0:97, :])
                nc.sync.dma_start(e_out[:, off:off + w], e_sb[0:97:32, :])

            # 3-stage software pipeline: L1(b) || L2(b-1) || L3(b-2) so
            # the in-order PE stream never waits on the tanh (ACT) chain.
            nblocks = len(blocks)
            for i in range(nblocks + 2):
                if i < nblocks:
                    emit_l1(i)
                if 0 <= i - 1 < nblocks:
                    emit_l2(i - 1)
                if 0 <= i - 2 < nblocks:
                    emit_l3(i - 2)

    nc.compile()
    _PROGRAM_CACHE[key] = nc
    return nc


def kernel(**inputs) -> np.ndarray:
    Gs = np.ascontiguousarray(np.asarray(inputs["Gs"], dtype=np.float32))
    types = np.asarray(inputs["types"])
    mol_id = np.asarray(inputs["mol_id"])
    n_atoms = Gs.shape[0]

    idx = [np.flatnonzero(types == 0), np.flatnonzero(types != 0)]
    # Per-core atom counts (equal across cores for SPMD; pad with zeros).
    GRAN = 128
    n_a, n_b = (int(-(-len(ix) // (N_CORES * GRAN))) * GRAN for ix in idx)
    npc = n_a + n_b

    GsT = Gs.astype(np.float16).T  # [128, N] fp16 view

    in_maps = []
    wk = {}
    for t, pre in (("a", "A"), ("b", "B")):
        wk[f"w1{t}"] = np.ascontiguousarray(
            np.asarray(inputs[f"W1_{pre}"], np.float32).astype(np.float16))
        wk[f"w2{t}"] = np.ascontiguousarray(
            np.asarray(inputs[f"W2_{pre}"], np.float32).astype(np.float16))
        w3chunks = np.asarray(
            inputs[f"W3_{pre}"], np.float32)[:, 0].reshape(MCH, 128).T
        w3p = np.zeros((128, 32 * MCH), np.float16)
        w3p[:, 0::32] = w3chunks.astype(np.float16)
        wk[f"w3{t}"] = w3p
        wk[f"b1{t}"] = np.ascontiguousarray(
            np.asarray(inputs[f"b1_{pre}"], np.float32).reshape(MCH, 128).T)
        wk[f"b2{t}"] = np.ascontiguousarray(
            np.asarray(inputs[f"b2_{pre}"], np.float32).reshape(MCH, 128).T)
        wk[f"b3{t}"] = np.float32(
            np.asarray(inputs[f"b3_{pre}"], np.float32).reshape(())
            + np.asarray(inputs[f"off_{pre}"], np.float32).reshape(()))

    chunks = []  # per core: (a_indices, b_indices)
    for i in range(N_CORES):
        ca = idx[0][i * n_a:(i + 1) * n_a]
        cb = idx[1][i * n_b:(i + 1) * n_b]
        chunks.append((ca, cb))
        buf = np.zeros((NUM_GS, npc), np.float16)
        buf[:, :len(ca)] = GsT[:, ca]
        buf[:, n_a:n_a + len(cb)] = GsT[:, cb]
        in_maps.append({"gst": buf,
                        **{k: v for k, v in wk.items()
                           if not k.startswith("b3")}})

    nc = _build_program(n_a, n_b)
    results = None
    for attempt in range(3):
        try:
            results = run_bass_kernel_spmd(
                nc, in_maps, list(range(N_CORES))).results
            break
        except Exception:
            # Transient NRT/device hiccups (e.g. NRT_EXEC_UNIT_UNRECOVERABLE)
            # usually clear on retry.
            if attempt == 2:
                raise
            time.sleep(2.0)

    e = np.empty(n_atoms, np.float32)
    for i in range(N_CORES):
        r = np.asarray(results[i]["e_out"]).sum(axis=0, dtype=np.float32)
        ca, cb = chunks[i]
        e[ca] = r[:len(ca)] + wk["b3a"]
        e[cb] = r[n_a:n_a + len(cb)] + wk["b3b"]

    sums = np.bincount(mol_id, weights=e.astype(np.float64),
                       minlength=N_MOL)[:N_MOL]
    counts = np.bincount(mol_id, minlength=N_MOL)[:N_MOL]
    out = sums / np.maximum(counts, 1)
    return out.astype(np.float32)[:, None]
